# revision 33
# baseline (speedup 1.0000x reference)
"""Trainium2 Bass kernel for GQA attention (B=1, S=2048, D=2048, H=16, KVH=4, HD=128).

Measured cost model of this environment (microbenchmarked):
  ~15-20us per *executed* instruction, +~40-70us extra for each *unique*
  instruction (first execution); loop iterations of plain nested static
  For_i are cheap (~13us); For_i_pipelined / staggered_reset / manual
  multi-buffering / dynamic bounds all multiply unique-instruction cost
  and are avoided entirely.  PE matmul FLOPs and DMA payload bytes are
  nearly free; collectives pipeline to ~0 incremental.

Design (vs the previous 36ms/24ms baseline):
  - Tensor parallel over heads (core c: q-heads {2c, 2c+1}, kv-head c//2);
    per-core partial wo outputs in bf16, summed on the host (unshard of a
    partial-sum sharding; replaces the device ReduceScatter).
  - Plain nested static For_i everywhere; every loop body stages its
    matmul stationary operand once (DVE copy with register-offset source)
    and packs the maximum number of matmuls per staged tile that PSUM
    allows (8 banks), minimizing both executed and unique instructions.
  - Phase 1 (QKV) accumulates q0|q1|kT|vT over 16 contraction tiles into
    all 8 PSUM banks, 2 S-chunks per pass, one 4-slab PSUM->SBUF copy per
    pass.  RoPE head-dim permutation trick: wq/wk columns permuted per
    head to [even|odd] so RoPE is two contiguous 64-partition halves.
  - Phase 1b transposes vT->v in place (chunk-diagonal, staged first).
  - RoPE straight-line with the three slabs' dependency chains
    interleaved so DMA/DVE latencies overlap.
  - Phase 2: uniform 4x4 (qc x grp) nested loop, causality via a 7-variant
    additive mask table indexed by qc-grp+3 (register arithmetic with a
    negative coefficient); per-h PSUM separation decouples the PE/Act
    chains of the two heads; softmax denominators via DVE reduce +
    ones-matmul accumulated in PSUM.
  - Phase 3 loops over the 16 output row-tiles with attnT as the staged
    stationary and wo as the static moving operand, writing natural-
    orientation [S, D] bf16 partials straight to DRAM.
"""

import numpy as np
import ml_dtypes
from contextlib import ExitStack

import concourse.bacc as bacc
import concourse.bass as bass
import concourse.tile as tile
import concourse.mybir as mybir
from concourse.bass_utils import run_bass_kernel_spmd

S = 2048
D = 2048
H = 16
KVH = 4
HD = 128
NCORES = 8
F32 = mybir.dt.float32
BF16 = mybir.dt.bfloat16
NPBF16 = ml_dtypes.bfloat16
SCALE = float(1.0 / np.sqrt(HD))
NEG = -1e9

# pack column offsets (bf16 elements per partition)
OFF_WO = 0              # [128, 2*2048]   = 4096
OFF_COSSIN = 4096       # [128, 2*2048]   = 4096 (cos|sin, duplicated halves)
OFF_MASK = 8192         # [128, 7*2048]   = 14336
OFF_IDENT = 22528       # [128, 128]
PACKW = 22656
# wx pack: [half(2), dtpair(8), unit(2), (w 512 | x-half 1024)] = 49152
WXW = 49152

_BUILD_CACHE = {}


def _emit_body(nc, tc, io):
    mm = nc.tensor.matmul
    ds = bass.ds
    with ExitStack() as ctx:
        sb = ctx.enter_context(tc.tile_pool(name="sb", bufs=1))
        dram = ctx.enter_context(tc.tile_pool(name="dram", bufs=1, space="DRAM"))

        pack = sb.tile([128, PACKW], BF16, tag="pack")
        nc.sync.dma_start(out=pack[:], in_=io["pack"][:])
        wo2 = pack[:, OFF_WO:OFF_WO + 4096].rearrange("p (f n) -> p f n", f=2)
        cosv = pack[0:64, OFF_COSSIN:OFF_COSSIN + 2048]
        sinv = pack[0:64, OFF_COSSIN + 2048:OFF_COSSIN + 4096]
        maskv = pack[:, OFF_MASK:OFF_MASK + 14336]
        ident = pack[:, OFF_IDENT:OFF_IDENT + 128]

        ones_sb = sb.tile([128, 1], BF16, tag="ones")
        nc.vector.memset(ones_sb[:], 1.0)

        # persistent activations
        qkv_sb = sb.tile([128, 4, 2048], BF16, tag="qkv")   # q0|q1|kT|vT->v
        attnT = sb.tile([128, 2, 2048], BF16, tag="attnT")  # [hd, (h, q)]
        recip = sb.tile([1, 4096], F32, tag="recip")

        # ---------------- phase 1: QKV projections (transposed) --------------
        # wx interleaves the weight chunk with the x chunk it multiplies, so
        # each iteration stages ONE copy feeding 16 fully-static matmuls.
        with tc.tile_pool(name="xp", bufs=1) as xp, \
             tc.tile_pool(name="p1s", bufs=1) as p1s, \
             tc.tile_pool(name="pp1", bufs=1, space="PSUM") as pp1:
            wx = xp.tile([128, WXW], BF16, tag="wx")
            nc.sync.dma_start(out=wx[:], in_=io["wx"][:])
            wxst = p1s.tile([128, 6144], BF16, tag="wxst")
            psA = pp1.tile([128, 4, 2, 512], F32, tag="psA")  # 8 banks

            with tc.For_i(0, 2, 1) as half:
                nc.vector.memset(psA[:], 0.0)
                with tc.For_i(0, 4, 1) as dq:
                    nc.vector.tensor_copy(
                        wxst[:], wx[:, ds(half * 24576 + dq * 6144, 6144)])
                    for u in range(4):
                        for j in range(4):
                            for s2 in range(2):
                                mm(psA[:, j, s2, :],
                                   wxst[:, u * 1536 + j * 128:u * 1536 + (j + 1) * 128],
                                   wxst[:, u * 1536 + 512 + s2 * 512:
                                        u * 1536 + 512 + (s2 + 1) * 512],
                                   start=False, stop=False)
                nc.vector.tensor_copy(
                    qkv_sb[:, :, ds(half * 1024, 1024)],
                    psA[:].rearrange("p a b c -> p a (b c)"))

        # -------- phase 1b + 1c: v transposes overlapped with RoPE loads -----
        qk_lo = qkv_sb[0:64, :, :]
        qk_hi = qkv_sb[64:128, :, :]
        with tc.tile_pool(name="rp", bufs=1) as rp:
            t1c = [rp.tile([64, 2048], BF16, tag=f"t1c{j}", name=f"t1c{j}")
                   for j in range(3)]
            o1 = [rp.tile([64, 2048], BF16, tag=f"o1{j}", name=f"o1{j}")
                  for j in range(3)]
            tmp = [rp.tile([64, 2048], BF16, tag=f"tmp{j}", name=f"tmp{j}")
                   for j in range(3)]
            # issue the RoPE hi-half loads first: they only read q/k slabs,
            # so they overlap the slab-3 transpose loop below
            for j in range(3):
                nc.sync.dma_start(out=t1c[j][:], in_=qk_hi[:, j, :])

            with tc.tile_pool(name="p2s", bufs=1) as p2s, \
                 tc.tile_pool(name="pp2", bufs=1, space="PSUM") as pp2:
                tst = p2s.tile([128, 512], BF16, tag="tst")
                psT = pp2.tile([128, 4, 128], BF16, tag="psT")
                with tc.For_i(0, 4, 1) as kt:
                    nc.vector.tensor_copy(tst[:], qkv_sb[:, 3, ds(kt * 512, 512)])
                    for j in range(4):
                        nc.tensor.transpose(psT[:, j, :],
                                            tst[:, j * 128:(j + 1) * 128],
                                            ident)
                    nc.vector.tensor_copy(
                        qkv_sb[:, 3, ds(kt * 512, 512)],
                        psT[:].rearrange("p a b -> p (a b)"))

            for j in range(3):
                nc.vector.tensor_mul(o1[j][:], qk_lo[:, j, :], sinv)
            for j in range(3):
                nc.vector.tensor_mul(tmp[j][:], t1c[j][:], cosv)
            for j in range(3):
                nc.vector.tensor_add(o1[j][:], o1[j][:], tmp[j][:])
            for j in range(3):
                nc.vector.tensor_mul(tmp[j][:], t1c[j][:], sinv)
            for j in range(3):
                nc.vector.tensor_mul(t1c[j][:], qk_lo[:, j, :], cosv)
            for j in range(3):
                nc.vector.tensor_sub(qk_lo[:, j, :], t1c[j][:], tmp[j][:])
            for j in range(3):
                nc.sync.dma_start(out=qk_hi[:, j, :], in_=o1[j][:])

        # ---------------- phase 2: attention (uniform qc x grp) --------------
        with tc.tile_pool(name="p3s", bufs=1) as p3s, \
             tc.tile_pool(name="pp3", bufs=1, space="PSUM") as pp3:
            kvst = p3s.tile([128, 2, 1024], BF16, tag="kvst")
            qst = p3s.tile([128, 2, 512], BF16, tag="qst")
            probs0 = p3s.tile([128, 4, 512], BF16, tag="probs0")
            probs1 = p3s.tile([128, 4, 512], BF16, tag="probs1")
            probsv = (probs0, probs1)
            ps_sc = pp3.tile([128, 4, 512], F32, tag="ps_sc")
            pv01 = pp3.tile([128, 2, 512], F32, tag="pv01")
            den0 = pp3.tile([1, 512], F32, tag="den0")
            den1 = pp3.tile([1, 512], F32, tag="den1")
            dens = (den0, den1)

            with tc.For_i(0, 4, 1) as qc:
                nc.vector.memset(pv01[:], 0.0)
                nc.vector.memset(den0[:], 0.0)
                nc.vector.memset(den1[:], 0.0)
                nc.vector.tensor_copy(qst[:], qkv_sb[:, 0:2, ds(qc * 512, 512)])
                with tc.For_i(0, 2, 1) as g2:
                    nc.vector.tensor_copy(kvst[:],
                                          qkv_sb[:, 2:4, ds(g2 * 1024, 1024)])
                    for u in range(2):
                        for h in range(2):
                            for i in range(4):
                                mm(ps_sc[:, i, :],
                                   kvst[:, 0, u * 512 + i * 128:u * 512 + (i + 1) * 128],
                                   qst[:, h, :],
                                   start=True, stop=True)
                            nc.vector.tensor_add(
                                ps_sc[:].rearrange("p a b -> p (a b)"),
                                ps_sc[:].rearrange("p a b -> p (a b)"),
                                maskv[:, ds(qc * 2048 - g2 * 4096 + 6144 - u * 2048,
                                            2048)])
                            nc.scalar.activation(probsv[h][:], ps_sc[:],
                                                 mybir.ActivationFunctionType.Exp,
                                                 scale=SCALE)
                            for i in range(4):
                                mm(pv01[:, h, :],
                                   kvst[:, 1, u * 512 + i * 128:u * 512 + (i + 1) * 128],
                                   probsv[h][:, i, :], start=False, stop=False)
                            for i in range(4):
                                mm(dens[h][:], ones_sb[:], probsv[h][:, i, :],
                                   start=False, stop=False)
                nc.vector.tensor_copy(attnT[:, :, ds(qc * 512, 512)], pv01[:])
                for h in range(2):
                    nc.vector.reciprocal(
                        recip[:, ds(h * 2048 + qc * 512, 512)], dens[h][:])

        # normalize: DRAM-bounce broadcast of 1/den, one big multiply
        rb = dram.tile([1, 4096], F32, name="rb")
        nc.sync.dma_start(out=rb[:], in_=recip[:])
        with tc.tile_pool(name="bcp", bufs=1) as bcp:
            bc = bcp.tile([128, 4096], F32, tag="bc")
            nc.sync.dma_start(out=bc[:], in_=rb.to_broadcast((128, 4096)))
            nc.vector.tensor_mul(attnT[:].rearrange("p a b -> p (a b)"),
                                 attnT[:].rearrange("p a b -> p (a b)"), bc[:])

        # ---------------- phase 3: output projection (natural rows) ----------
        # out[qt*128+p, n] = sum_f attnT[f, h, qt*128+p] * wo[f(h), n]
        with tc.tile_pool(name="p4s", bufs=1) as p4s, \
             tc.tile_pool(name="pp4", bufs=1, space="PSUM") as pp4:
            ast = p4s.tile([128, 2, 256], BF16, tag="ast")
            osb = p4s.tile([128, 16, 2048], BF16, tag="osb")
            ps_o = pp4.tile([128, 2, 4, 512], F32, tag="ps_o")  # 8 banks
            with tc.For_i(0, 8, 1) as qt2:
                nc.vector.tensor_copy(ast[:], attnT[:, :, ds(qt2 * 256, 256)])
                for u in range(2):
                    for n4 in range(4):
                        for f in range(2):
                            mm(ps_o[:, u, n4, :],
                               ast[:, f, u * 128:(u + 1) * 128],
                               wo2[:, f, n4 * 512:(n4 + 1) * 512],
                               start=(f == 0), stop=(f == 1))
                nc.vector.tensor_copy(
                    osb[:].rearrange("p a b -> p (a b)")[:, ds(qt2 * 4096, 4096)],
                    ps_o[:].rearrange("p u a b -> p (u a b)"))
            # out rows = qt*128 + p: view out as [qt, p, n] -> [p, qt, n]
            nc.sync.dma_start(
                out=io["out"].rearrange("(a p) n -> p a n", p=128),
                in_=osb[:])


def build(repeat=1, num_devices=NCORES):
    key = (repeat, num_devices)
    if key in _BUILD_CACHE:
        return _BUILD_CACHE[key]
    nc = bacc.Bacc("TRN2", target_bir_lowering=False, debug=False,
                   num_devices=num_devices)
    io = {
        "pack": nc.dram_tensor("pack", [128, PACKW], BF16,
                               kind="ExternalInput").ap(),
        "wx": nc.dram_tensor("wx", [128, WXW], BF16,
                             kind="ExternalInput").ap(),
        "out": nc.dram_tensor("out", [2048, 2048], BF16,
                              kind="ExternalOutput").ap(),
    }
    with tile.TileContext(nc) as tc:
        for _ in range(repeat):
            _emit_body(nc, tc, io)
    nc.compile()
    _BUILD_CACHE[key] = nc
    return nc


def prepare_in_maps(x, wq, wk, wv, wo, freqs_cos, freqs_sin):
    bf = lambda a: np.ascontiguousarray(a).astype(NPBF16)
    x2d = np.asarray(x, dtype=np.float32).reshape(S, D)
    # xT tiled: [128, 16, 2048], row p tile dt <-> input dim dt*128+p
    xT = bf(x2d.T.reshape(16, 128, S).transpose(1, 0, 2).reshape(128, 32768))

    cosT = np.asarray(freqs_cos, np.float32).T                # [64, S]
    sinT = np.asarray(freqs_sin, np.float32).T
    cossin64 = np.concatenate([cosT, sinT], axis=1)           # [64, 4096]
    cossin = np.concatenate([cossin64, cossin64], axis=0)     # [128, 4096] dup

    perm = np.concatenate([np.arange(0, HD, 2), np.arange(1, HD, 2)])
    wq = np.asarray(wq, np.float32)
    wk = np.asarray(wk, np.float32)
    wv = np.asarray(wv, np.float32)
    wo = np.asarray(wo, np.float32)
    wqP = wq.reshape(D, H, HD)[:, :, perm]                    # [D, 16, 128]
    wkP = wk.reshape(D, KVH, HD)[:, :, perm]                  # [D, 4, 128]
    wv4 = wv.reshape(D, KVH, HD)

    # mask variant table [kl, v, ql]: v = qc-grp+3; v<3 -> NEG (non-causal
    # group), v==3 -> boundary quad, v>3 -> 0 (fully causal group)
    kl = np.arange(128)[:, None, None]
    iv = np.arange(4)[None, :, None]
    qlv = np.arange(512)[None, None, :]
    maskB = np.where(128 * iv + kl <= qlv, 0.0, NEG).reshape(128, 2048)
    maskv = np.zeros((128, 7, 2048), np.float32)
    maskv[:, 0:3, :] = NEG
    maskv[:, 3, :] = maskB
    maskv = maskv.reshape(128, 14336)

    ident = np.eye(128, dtype=np.float32)

    xT3 = xT.astype(np.float32).reshape(128, 16, 2048)
    in_maps = []
    for c in range(NCORES):
        g = c // 2
        # wqkv packed [p, dt*512 + (q0|q1|k|v)*128 + col]
        wqkv = np.stack([wqP[:, 2 * c, :], wqP[:, 2 * c + 1, :],
                         wkP[:, g, :], wv4[:, g, :]], axis=1)  # [D, 4, 128]
        wqkv = wqkv.reshape(16, 128, 512).transpose(1, 0, 2)  # [128, 16, 512]
        # wx interleave: [p, half, dtquad, unit(4), (w 512 | x-half 1024)]
        wx = np.empty((128, 2, 4, 4, 1536), np.float32)
        for half in range(2):
            for u in range(4):
                wx[:, half, :, u, 0:512] = wqkv[:, u::4, :]
                wx[:, half, :, u, 512:1536] = \
                    xT3[:, u::4, half * 1024:(half + 1) * 1024]
        wx = wx.reshape(128, WXW)
        # wo rows for this core's heads: [p, f(2), n(2048)]
        woc = wo[256 * c:256 * c + 256, :]                    # [256, 2048]
        wo2 = woc.reshape(2, 128, 2048).transpose(1, 0, 2).reshape(128, 4096)
        packc = np.concatenate(
            [wo2, cossin, maskv, ident], axis=1)              # [128, PACKW]
        assert packc.shape == (128, PACKW), packc.shape
        in_maps.append({"pack": bf(packc), "wx": bf(wx)})
    return in_maps


def assemble_output(results):
    acc = np.zeros((S, D), np.float32)
    for c in range(NCORES):
        o = np.asarray(results[c]["out"], np.float32)         # [2048, 2048]
        acc += o
    return np.ascontiguousarray(acc).reshape(1, S, D)


def kernel(x, wq, wk, wv, wo, freqs_cos, freqs_sin, mask):
    nc = build()
    in_maps = prepare_in_maps(x, wq, wk, wv, wo, freqs_cos, freqs_sin)
    res = run_bass_kernel_spmd(nc, in_maps, core_ids=list(range(NCORES)))
    return assemble_output(res.results).astype(np.float32)
